# revision 22
# baseline (speedup 1.0000x reference)
"""Attention kernel for trn2: B=4, N=2048, DIM=512, HEADS=8, DIM_HEAD=64.

Sharding: head-parallel across 8 cores (core h computes head h for all 4
batches). Each core returns a partial [4, 2048, 512] bf16 output (its head's
contribution through W_out); the host sums the 8 partials in fp32.

Per-core pipeline (all matmuls bf16, fp32 PSUM accumulate):
  phase 1 (projections, W-stationary so q/k emerge pre-transposed):
    QKc^T = [Wq|Wk]^T x^T and QKs^T = [Wq P|Wk P]^T x^T  (P = rotate-half
    permutation folded into the weights on host), then rotary is just
    rot = QKc*cos + QKs*sin on DVE (position runs along the free axis).
    v is projected x-stationary into natural [n, d] layout. DMA sbuf->sbuf
    remaps build qdup (q^T duplicated into both partition halves) and kTp
    (k^T chunks packed by parity into halves).
  phase 2 (attention, per (batch, 512-wide q tile)):
    S^T pairs via two concurrent K=64 row-tiled matmuls -> 2 psum banks;
    ACT exp over the [128,1024] pair (psum->sbuf bf16); attn = et * expB
    (host-precomputed exp(bias^T) bf16, loaded once per q-tile and shared
    by all 4 batches) on DVE; PV accumulates out^T (+ ones column
    for the softmax denominator); denominator is transposed via K=1
    matmuls to get per-partition reciprocals; W_out projection (K=64) with
    normalization folded into the psum evacuation as a tensor_scalar mul.
"""

import numpy as np

B, N, DIM = 4, 2048, 512
HEADS, DH = 8, 64
P = 128
DC = DIM // P          # 4 dim chunks
NCH = N // P           # 16 n chunks
QT = 512               # q tile in phase 2
NQT = N // QT          # 4
PAIRS = NCH // 2       # 8 k-chunk pairs
NB = N // QT           # 4 n blocks in phase 1

_CACHE = {}


def _build():
    import concourse.mybir as mybir
    import concourse.tile as tile
    from concourse import bacc

    F32 = mybir.dt.float32
    BF16 = mybir.dt.bfloat16
    EXP = mybir.ActivationFunctionType.Exp

    nc = bacc.Bacc(None, target_bir_lowering=False)

    # ---- inputs ----
    xT4_d = nc.dram_tensor("xT4", [B, P, DC, N], BF16, kind="ExternalInput")
    wqk_d = nc.dram_tensor("wqk", [P, 2, DC, P], BF16, kind="ExternalInput")
    wv_d = nc.dram_tensor("wv", [P, DC, DH], BF16, kind="ExternalInput")
    wout_d = nc.dram_tensor("wout", [DH, DIM], BF16, kind="ExternalInput")
    expb_d = nc.dram_tensor(
        "expb", [NQT, P, PAIRS, 2, QT], BF16, kind="ExternalInput"
    )
    cos2_d = nc.dram_tensor("cos2", [P, N], BF16, kind="ExternalInput")
    sin2_d = nc.dram_tensor("sin2", [P, N], BF16, kind="ExternalInput")
    onesv_d = nc.dram_tensor("onesv", [P, NCH], BF16, kind="ExternalInput")
    vinit_d = nc.dram_tensor(
        "vinit", [P, NCH, DH + 1], BF16, kind="ExternalInput"
    )
    out_d = nc.dram_tensor("out", [B, N, DIM], BF16, kind="ExternalOutput")

    with tile.TileContext(nc) as tc:
        with tc.tile_pool(name="const", bufs=1) as cp:
            wqk_t = cp.tile([P, 2, DC, P], BF16, tag="wqk")
            nc.sync.dma_start(wqk_t[:], wqk_d[:, :, :, :])
            wv_t = cp.tile([P, DC, DH], BF16, tag="wv")
            nc.sync.dma_start(wv_t[:], wv_d[:, :, :])
            wout_t = cp.tile([DH, DIM], BF16, tag="wout")
            nc.sync.dma_start(wout_t[:], wout_d[:, :])
            cos2_t = cp.tile([P, N], BF16, tag="cos2")
            nc.scalar.dma_start(cos2_t[:], cos2_d[:, :])
            sin2_t = cp.tile([P, N], BF16, tag="sin2")
            nc.scalar.dma_start(sin2_t[:], sin2_d[:, :])
            ones_t = cp.tile([P, NCH], BF16, tag="ones")
            nc.scalar.dma_start(ones_t[:], onesv_d[:, :])

            # persistent per-batch activations
            qdup_b = [cp.tile([P, N], BF16, tag=f"qdup{b}", name=f"qdup{b}") for b in range(B)]
            kTp_b = [cp.tile([P, PAIRS, P], BF16, tag=f"kTp{b}", name=f"kTp{b}") for b in range(B)]
            v_b = [cp.tile([P, NCH, DH + 1], BF16, tag=f"v{b}", name=f"v{b}") for b in range(B)]
            for b in range(B):
                nc.scalar.dma_start(v_b[b][:], vinit_d[:, :, :])

            # ---- fused phase 1 + phase 2 ----
            # phase1(b) is emitted, then phase2(jq=0, b) immediately after, so
            # the scheduler hides projections for batches 1-3 under the
            # ACT-bound attention pipeline of earlier batches.
            with (
                tc.tile_pool(name="p1", bufs=3) as p1,
                tc.tile_pool(name="p1x", bufs=5) as p1x,
                tc.tile_pool(name="eb", bufs=2) as ebp,
                tc.tile_pool(name="p2", bufs=3) as p2,
                tc.tile_pool(name="psA", bufs=2, space="PSUM") as psA,
                tc.tile_pool(name="psB", bufs=2, space="PSUM") as psB,
            ):

                def phase1(b):
                    rot = p1.tile([P, N], BF16, tag="rot", name=f"rot{b}")
                    for nb in range(NB):
                        ns = slice(nb * QT, (nb + 1) * QT)
                        xblk = p1x.tile([P, DC, QT], BF16, tag="xblk", name="xblk")
                        nc.sync.dma_start(xblk[:], xT4_d[b, :, :, ns])
                        qk_ps = psA.tile([P, 2, QT], F32, tag="s", name="qk_ps")
                        for g in range(2):
                            for dc in range(DC):
                                nc.tensor.matmul(
                                    qk_ps[:, g],
                                    lhsT=wqk_t[:, g, dc],
                                    rhs=xblk[:, dc],
                                    start=(dc == 0),
                                    stop=(dc == DC - 1),
                                )
                        vtile = psB.tile([P, DIM], F32, tag="wo", name="vblk")
                        vblk_ps = vtile[:, 0 : 4 * DH].rearrange(
                            "p (a b) -> p a b", a=4
                        )
                        for ci in range(4):
                            for dc in range(DC):
                                nc.tensor.matmul(
                                    vblk_ps[:, ci],
                                    lhsT=xblk[:, dc, ci * P : (ci + 1) * P],
                                    rhs=wv_t[:, dc],
                                    start=(dc == 0),
                                    stop=(dc == DC - 1),
                                )
                        nc.vector.tensor_copy(
                            v_b[b][:, nb * 4 : nb * 4 + 4, 0:DH], vblk_ps[:]
                        )
                        # rotary: rot = qkc*cos + qks*sin (muls read psum direct)
                        m1 = p1.tile([P, QT], BF16, tag="m1", name="m1")
                        nc.vector.tensor_mul(m1[:], qk_ps[:, 0], cos2_t[:, ns])
                        m2 = p1.tile([P, QT], BF16, tag="m2", name="m2")
                        nc.vector.tensor_mul(m2[:], qk_ps[:, 1], sin2_t[:, ns])
                        nc.vector.tensor_add(rot[:, ns], m1[:], m2[:])
                    # layout remaps via DMA on the scalar-engine queue so
                    # they don't block the next batch's x loads on sync
                    nc.scalar.dma_start(qdup_b[b][0:DH, :], rot[0:DH, :])
                    nc.scalar.dma_start(qdup_b[b][DH:P, :], rot[0:DH, :])
                    nc.scalar.dma_start(
                        kTp_b[b][0:DH, :, :], rot[DH:P, 0 : PAIRS * P]
                    )
                    nc.scalar.dma_start(
                        kTp_b[b][DH:P, :, :], rot[DH:P, PAIRS * P : N]
                    )

                def pe_tail(prev):
                    hod, jq, b = prev
                    dT_ps = psB.tile([P, DIM], F32, tag="wo", name="dTw")
                    for s4 in range(4):
                        nc.tensor.matmul(
                            dT_ps[:, s4 : s4 + 1],
                            lhsT=hod[DH : DH + 1, s4 * P : (s4 + 1) * P],
                            rhs=ones_t[DH : DH + 1, 0:1],
                            start=True,
                            stop=True,
                        )
                    wo_list = []
                    for sq in range(4):
                        wo_ps = psB.tile([P, DIM], F32, tag="wo", name="wo")
                        nc.tensor.matmul(
                            wo_ps[:],
                            lhsT=hod[0:DH, sq * P : (sq + 1) * P],
                            rhs=wout_t[:],
                            start=True,
                            stop=True,
                        )
                        wo_list.append(wo_ps)
                    return dT_ps, wo_list

                def dve_tail(prev, dT_ps, wo_list):
                    hod, jq, b = prev
                    rs = p2.tile([P, 4], F32, tag="rs", name="rs")
                    with nc.allow_low_precision(reason="softmax recip"):
                        nc.vector.reciprocal(rs[:], dT_ps[:, 0:4])
                    for sq in range(4):
                        ob = p2.tile([P, DIM], BF16, tag="ob", name="ob")
                        nc.vector.tensor_scalar_mul(
                            ob[:], wo_list[sq][:], rs[:, sq : sq + 1]
                        )
                        row0 = jq * QT + sq * P
                        nc.sync.dma_start(out_d[b, row0 : row0 + P, :], ob[:])

                def phase2_body(jq, b, eb_t, prev):
                    qs = slice(jq * QT, (jq + 1) * QT)
                    outT_ps = psB.tile([DH + 1, QT], F32, tag="outT", name="outT")
                    n_pv = 0
                    total_pv = 2 * PAIRS
                    tail_state = None
                    for g in range(PAIRS // 2):
                        et4 = p2.tile([P, 2, 2, QT], BF16, tag="et4", name="et4")
                        attn4 = p2.tile(
                            [P, 2, 2, QT], BF16, tag="attn4", name="attn4"
                        )
                        for h in range(2):
                            pr = 2 * g + h
                            s_ps = psA.tile([P, 2, QT], F32, tag="s", name="s_ps")
                            nc.tensor.matmul(
                                s_ps[:, 0],
                                lhsT=kTp_b[b][0:DH, pr],
                                rhs=qdup_b[b][0:DH, qs],
                                start=True,
                                stop=True,
                                tile_position=(0, 0),
                            )
                            nc.tensor.matmul(
                                s_ps[:, 1],
                                lhsT=kTp_b[b][DH:P, pr],
                                rhs=qdup_b[b][DH:P, qs],
                                start=True,
                                stop=True,
                                tile_position=(64, 0),
                            )
                            nc.scalar.activation(et4[:, h], s_ps[:], EXP)
                        if g == 1 and prev is not None:
                            # PE tail of the previous iteration: emitted here so
                            # the boundary has no PE work between PV(g3) and
                            # S(g0), and late enough that ACT stays fed
                            tail_state = pe_tail(prev)
                        nc.vector.tensor_mul(
                            attn4[:], et4[:], eb_t[:, 2 * g : 2 * g + 2]
                        )
                        for h in range(2):
                            pr = 2 * g + h
                            for j in range(2):
                                n_pv += 1
                                nc.tensor.matmul(
                                    outT_ps[:],
                                    lhsT=v_b[b][:, pr + PAIRS * j],
                                    rhs=attn4[:, h, j],
                                    start=(n_pv == 1),
                                    stop=(n_pv == total_pv),
                                )
                    # evacuate out^T + denominator row in one copy
                    hod = p2.tile([DH + 1, QT], BF16, tag="hod", name="hod")
                    nc.vector.tensor_copy(hod[:], outT_ps[:])
                    # DVE tail of the previous iteration runs after this
                    # iteration's muls so PV feeding is never delayed
                    if prev is not None:
                        dve_tail(prev, *tail_state)
                    return (hod, jq, b)

                for b in range(B):
                    phase1(b)
                prev = None
                for jq in range(NQT):
                    eb_t = ebp.tile([P, PAIRS, 2, QT], BF16, tag="eb", name="eb")
                    nc.sync.dma_start(eb_t[:], expb_d[jq])
                    for b in range(B):
                        prev = phase2_body(jq, b, eb_t, prev)
                final_state = pe_tail(prev)
                dve_tail(prev, *final_state)

    nc.compile()
    return nc


def _host_inputs(x, pos_bias, W_qkv, W_out):
    """Build the per-core input maps (pure data marshalling)."""
    import ml_dtypes

    bf16 = ml_dtypes.bfloat16

    xT = np.ascontiguousarray(x.transpose(0, 2, 1))          # [B, DIM, N]
    xT4 = np.ascontiguousarray(
        xT.reshape(B, DC, P, N).transpose(0, 2, 1, 3)
    ).astype(bf16)                                           # [B, P, DC, N]

    # split-d permutation: even dims then odd dims
    perm = np.concatenate([np.arange(0, DH, 2), np.arange(1, DH, 2)])
    inv_freq = (1.0 / (10000.0 ** (np.arange(0, DH, 2, dtype=np.float32) / DH)))
    pos = np.arange(N, dtype=np.float32)
    fr = inv_freq[:, None] * pos[None, :]                     # [32, N]
    cos_h = np.cos(fr)
    sin_h = np.sin(fr)
    # rows: q-even, q-odd, k-even, k-odd halves all share the per-pair angle
    cos2 = np.concatenate([cos_h] * 4, axis=0).astype(bf16)   # [128, N]
    sin2 = np.concatenate([sin_h] * 4, axis=0).astype(bf16)

    onesv = np.ones((P, NCH), dtype=np.float32).astype(bf16)
    vinit = np.zeros((P, NCH, DH + 1), dtype=np.float32)
    vinit[:, :, DH] = 1.0
    vinit = vinit.astype(bf16)

    scale = np.float32(DH ** -0.5)
    in_maps = []
    for h in range(HEADS):
        Wq = (W_qkv[:, h * DH : (h + 1) * DH] * scale)[:, perm]   # split-d
        Wk = W_qkv[:, DIM + h * DH : DIM + (h + 1) * DH][:, perm]
        Wv = W_qkv[:, 2 * DIM + h * DH : 2 * DIM + (h + 1) * DH]
        # rotate-half in split layout: s_e = -c_o, s_o = c_e
        Wq_s = np.concatenate([-Wq[:, 32:64], Wq[:, 0:32]], axis=1)
        Wk_s = np.concatenate([-Wk[:, 32:64], Wk[:, 0:32]], axis=1)
        Wc = np.concatenate([Wq, Wk], axis=1)                 # [512, 128]
        Ws = np.concatenate([Wq_s, Wk_s], axis=1)             # [512, 128]
        wqk = np.ascontiguousarray(
            np.stack(
                [
                    Wc.reshape(DC, P, P).transpose(1, 0, 2),
                    Ws.reshape(DC, P, P).transpose(1, 0, 2),
                ],
                axis=1,
            )
        ).astype(bf16)                                        # [P, 2, DC, P]
        wv = np.ascontiguousarray(
            Wv.reshape(DC, P, DH).transpose(1, 0, 2)
        ).astype(bf16)                                        # [P, DC, DH]
        wout = np.ascontiguousarray(W_out[h * DH : (h + 1) * DH, :]).astype(bf16)
        ebT = np.exp(pos_bias[h].T.astype(np.float32))        # [k, q]
        expb = np.ascontiguousarray(
            ebT.reshape(2, PAIRS, P, NQT, QT).transpose(3, 2, 1, 0, 4)
        ).astype(bf16)                                        # [NQT, P, PAIRS, 2, QT]
        in_maps.append(
            {
                "xT4": xT4,
                "wqk": wqk,
                "wv": wv,
                "wout": wout,
                "expb": expb,
                "cos2": cos2,
                "sin2": sin2,
                "onesv": onesv,
                "vinit": vinit,
            }
        )
    return in_maps


def kernel(x, pos_bias, W_qkv, W_out, _trace=False):
    from concourse.bass_utils import run_bass_kernel_spmd

    x = np.asarray(x, dtype=np.float32)
    pos_bias = np.asarray(pos_bias, dtype=np.float32)
    W_qkv = np.asarray(W_qkv, dtype=np.float32)
    W_out = np.asarray(W_out, dtype=np.float32)

    if "nc" not in _CACHE:
        _CACHE["nc"] = _build()
    nc = _CACHE["nc"]

    in_maps = _host_inputs(x, pos_bias, W_qkv, W_out)
    try:
        res = run_bass_kernel_spmd(
            nc, in_maps, core_ids=list(range(HEADS)), trace=_trace
        )
    except ModuleNotFoundError:
        res = run_bass_kernel_spmd(
            nc, in_maps, core_ids=list(range(HEADS)), trace=False
        )
    out = np.zeros((B, N, DIM), dtype=np.float32)
    for rmap in res.results:
        out += rmap["out"].astype(np.float32)
    if _trace:
        return out, res
    return out


if __name__ == "__main__":
    rng = np.random.default_rng(0)
    x = rng.standard_normal((B, N, DIM), dtype=np.float32)
    pb = rng.standard_normal((HEADS, N, N), dtype=np.float32)
    wq = rng.standard_normal((DIM, 3 * DIM), dtype=np.float32) * DIM**-0.5
    wo = rng.standard_normal((DIM, DIM), dtype=np.float32) * DIM**-0.5
    o = kernel(x, pb, wq, wo)
    print("kernel ran, out std:", o.std())
